# revision 31
# baseline (speedup 1.0000x reference)
"""Trainium2 Bass kernel for nn_BayesianLayer (dense_mlp).

Reference computation (B=32, R=2, IN=OUT=1024):
    sigma      = softplus(ro)                      # (IN, OUT)
    weights    = eps * sigma + mu                  # (B, R, IN, OUT)
    bias       = eps_b * softplus(ro_b) + mu_b     # (B, R, OUT)
    log_prior  = (mean(ln(mix(weights))) + mean(ln(mix(bias)))) / (B*R)
    log_p      = (mean(ln N(w; mu, sigma)) + mean(ln N(bias; mu_b, sigma_b))) / (B*R)
    out        = (einsum('bi,brio->bro', x, weights) + bias).mean(axis=1)

Device math (per element, w = eps*sigma + mu, q = w^2):
    ln mix(w)      = -q/8 - ln(sqrt(2pi)) + ln(0.5*exp(-3q/8) + 0.25)
    ln N(w;mu,sig) = -eps^2/2 - ln(sigma) - ln(sqrt(2pi))
Each core produces partial sums (Sum w^2, Sum ln(0.5e+0.25), Sum eps^2,
Sum ln sigma; same four for the bias) plus its 4 rows of `out`; the host
does the final exact float64 combination (the "all-reduce" of the hint).

Sharding: data-parallel over batch. Core c handles batches [4c, 4c+4).

Engine split per half-slab unit (128 x 4096 elements):
    DVE:    s = eps*sigma (f32r out) ; w = s + mu (f32r out)
    GpSimd: q = w*w (separate tile, rotates with s)
    ACT:    e = exp(-3q/8) in place over q ; ln(0.5e+0.25) -> dummy, +accum
    PE:     matvec psum += x @ s (f32r); Gram(w) and Gram(eps) via f32r
            matmuls into two accumulating (128,256) psum tiles whose tiled
            diagonals hold per-column Sum w^2 / Sum eps^2 (extracted at the
            end with a host-provided tiled-identity mask);
            x @ mu once in exact fp32 (M=4), injected into the bias row via
            an accumulating GpSimd DMA.
"""

import sys

for _p in ("/opt/trn_rl_repo",):
    if _p not in sys.path:
        sys.path.insert(0, _p)

import math
from contextlib import ExitStack

import numpy as np

import concourse.bacc as bacc
import concourse.bass as bass
import concourse.mybir as mybir
import concourse.tile as tile

f32 = mybir.dt.float32
f32r = mybir.dt.float32r
AF = mybir.ActivationFunctionType
OP = mybir.AluOpType

N_CORES = 8
LOG_SQRT_2PI = 0.5 * math.log(2.0 * math.pi)

# Stats layout (partition index of the [16,1] stats output)
S_QW, S_LGW, S_SQW, S_LSW, S_QB, S_LGB, S_SQB, S_LSB = range(8)


def build_kernel(IN=1024, OUT=1024, BPC=4, R=2, HT=4, gram_n=256, mv_f32r=True):
    """Per-core DRAM inputs:
      eps      [BPC*R, IT, 128, OUT] (f32r bits = raw f32)
      xT       [128, IT*BPC] (f32r)     col t*BPC+b holds x[b, t*128+p]
      mu, ro   [IT, 128, OUT]           (mu declared f32r)
      mu_bias, ro_bias [1, OUT]
      eps_bias [128, BRF]               flat (b, o, r) order
      diagmask [128, gram_n]            tiled identity for Gram diag extract
    Outputs: out [1, BPC*OUT] ; stats [16, 1]
    """
    IT = IN // 128
    assert IT % HT == 0
    NH = IT // HT
    OH = max(1, OUT // 512)
    ON = OUT // OH
    U = BPC * R * NH
    HF = HT * OUT
    GN = gram_n                  # Gram rhs width (>=256 keeps f32r at 1 cyc/row)
    GK = GN // 128               # 128-chunks per Gram group

    RPB = 128 // BPC
    OPR = OUT // RPB
    BRF = OPR * R

    nc = bacc.Bacc("TRN2", target_bir_lowering=False, debug=False,
                   num_devices=N_CORES)

    _c = nc.alloc_sbuf_tensor("const-f32-qmix", [128, 1], f32)
    nc.gpsimd.memset(_c.ap(), 0.25)
    nc.const_aps.aps[(f32, 0.25)] = _c.ap()
    nc.all_engine_barrier()

    eps_d = nc.dram_tensor("eps", [BPC * R, IT, 128, OUT], f32r, kind="ExternalInput")
    xT_d = nc.dram_tensor("xT", [128, IT * BPC], f32r, kind="ExternalInput")
    xTf_d = nc.dram_tensor("xTf", [128, IT * BPC], f32, kind="ExternalInput")
    mu_d = nc.dram_tensor("mu", [IT, 128, OUT], f32, kind="ExternalInput")
    ro_d = nc.dram_tensor("ro", [IT, 128, OUT], f32, kind="ExternalInput")
    mub_d = nc.dram_tensor("mu_bias", [1, OUT], f32, kind="ExternalInput")
    rob_d = nc.dram_tensor("ro_bias", [1, OUT], f32, kind="ExternalInput")
    epsb_d = nc.dram_tensor("eps_bias", [128, BRF], f32, kind="ExternalInput")
    mask_d = nc.dram_tensor("diagmask", [128, GN], f32, kind="ExternalInput")

    out_d = nc.dram_tensor("out", [1, BPC * OUT], f32, kind="ExternalOutput")
    stats_d = nc.dram_tensor("stats", [16, 1], f32, kind="ExternalOutput")

    with tile.TileContext(nc) as tc, ExitStack() as ctx:
        const = ctx.enter_context(tc.tile_pool(name="const", bufs=1))
        sigmu = ctx.enter_context(tc.tile_pool(name="sigmu", bufs=1))
        epsp = ctx.enter_context(tc.tile_pool(name="epsp", bufs=2))
        sqp = ctx.enter_context(tc.tile_pool(name="sqp", bufs=3))
        chA = ctx.enter_context(tc.tile_pool(name="chA", bufs=2))
        psmv = ctx.enter_context(tc.tile_pool(name="psmv", bufs=2, space="PSUM"))
        psg = ctx.enter_context(tc.tile_pool(name="psg", bufs=1, space="PSUM"))
        pssc = ctx.enter_context(tc.tile_pool(name="pssc", bufs=1, space="PSUM"))

        # ---------------- persistent tiles ----------------
        sig = sigmu.tile([128, IT * OUT], f32)
        mu = sigmu.tile([128, IT * OUT], f32)

        acclg = const.tile([128, U], f32)
        accls = const.tile([128, 2], f32)
        acc_stack = const.tile([128, 8], f32)
        for _t in (acclg, accls, acc_stack):
            nc.vector.memset(_t[:, :], 0.0)

        xT = const.tile([128, IT * BPC], f32r)
        xTf = const.tile([128, IT * BPC], f32)
        ones = const.tile([128, 1], f32)
        nc.vector.memset(ones[:, :], 1.0)
        dummy = const.tile([128, 1], f32)
        mask = const.tile([128, GN], f32)
        nc.sync.dma_start(mask[:, :], mask_d.ap())

        biash0 = const.tile([1, BPC * OUT], f32)  # 0.5*bias_sum + x@mu, then out
        xmu_sb = const.tile([BPC, OUT], f32)
        stats_sb = const.tile([16, 1], f32)

        nc.sync.dma_start(xT[:, :], xT_d.ap())
        nc.sync.dma_start(xTf[:, :], xTf_d.ap())

        # Gram accumulators
        gram_w = psg.tile([128, GN], f32)
        gram_e = psg.tile([128, GN], f32)

        # ---------------- mu / sigma setup ----------------
        nc.sync.dma_start(
            mu[:, :].rearrange("p (t o) -> p t o", t=IT),
            mu_d.ap().rearrange("t p o -> p t o"),
        )
        for m in range(NH):
            sl = slice(m * HF, (m + 1) * HF)
            nc.sync.dma_start(
                sig[:, sl].rearrange("p (t o) -> p t o", t=HT),
                ro_d.ap()[m * HT : (m + 1) * HT].rearrange("t p o -> p t o"),
            )
            tmp = chA.tile([128, HF], f32, tag="chA", name="tmp")
            nc.scalar.activation(tmp[:, :], sig[:, sl], AF.Exp)
            nc.scalar.activation(sig[:, sl], tmp[:, :], AF.Ln, bias=1.0, scale=1.0)
        for m in range(2):
            sl = slice(m * (IT * OUT // 2), (m + 1) * (IT * OUT // 2))
            nc.scalar.activation(
                dummy[:, :].to_broadcast((128, IT * OUT // 2)),
                sig[:, sl], AF.Ln, accum_out=accls[:, m : m + 1],
            )

        # ---------------- x @ mu (exact fp32, M=BPC) ----------------
        ps_xmu = psmv.tile([BPC, OUT], f32, tag="mv", name="ps_xmu")
        for t in range(IT):
            for oh in range(OH):
                nc.tensor.matmul(
                    ps_xmu[:, oh * ON : (oh + 1) * ON],
                    lhsT=xTf[:, t * BPC : (t + 1) * BPC],
                    rhs=mu[:, t * OUT + oh * ON : t * OUT + (oh + 1) * ON],
                    start=(t == 0),
                    stop=(t == IT - 1),
                )
        nc.scalar.copy(xmu_sb[:, :], ps_xmu[:, :])

        # ---------------- bias chain ----------------
        epsb = const.tile([128, BRF], f32)
        sigb = const.tile([128, OPR], f32)
        mub = const.tile([128, OPR], f32)
        biasb = const.tile([128, BRF], f32)
        bscr = const.tile([128, BRF], f32)
        bscr2 = const.tile([128, BRF], f32)
        biash = const.tile([128, BRF // 2], f32)
        lsb_acc = const.tile([128, 1], f32)
        qb_acc = const.tile([128, 1], f32)
        lgb_acc = const.tile([128, 1], f32)
        sqb_acc = const.tile([128, 1], f32)
        for t_ in (lsb_acc, qb_acc, lgb_acc, sqb_acc):
            nc.vector.memset(t_[:, :], 0.0)
        nc.sync.dma_start(epsb[:, :], epsb_d.ap())
        for b in range(BPC):
            nc.sync.dma_start(
                sigb[b * RPB : (b + 1) * RPB, :],
                rob_d.ap().rearrange("one (c v) -> (one c) v", c=RPB),
            )
            nc.sync.dma_start(
                mub[b * RPB : (b + 1) * RPB, :],
                mub_d.ap().rearrange("one (c v) -> (one c) v", c=RPB),
            )
        nc.scalar.activation(bscr[:, 0:OPR], sigb[:, :], AF.Exp)
        nc.scalar.activation(sigb[:, :], bscr[:, 0:OPR], AF.Ln, bias=1.0, scale=1.0)
        lnsb_out = const.tile([128, OPR], f32)
        nc.scalar.activation(
            lnsb_out[0:RPB, :], sigb[0:RPB, :], AF.Ln,
            accum_out=lsb_acc[0:RPB, 0:1],
        )
        sigb_b = sigb[:, :].to_broadcast((128, OPR, R))
        mub_b = mub[:, :].to_broadcast((128, OPR, R))
        epsb_3 = epsb[:, :].rearrange("p (v d) -> p v d", d=R)
        biasb_3 = biasb[:, :].rearrange("p (v d) -> p v d", d=R)
        nc.vector.tensor_tensor(biasb_3, epsb_3, sigb_b, OP.mult)
        nc.vector.tensor_tensor(biasb_3, biasb_3, mub_b, OP.add)
        nc.scalar.activation(bscr[:, :], biasb[:, :], AF.Square,
                             accum_out=qb_acc[:, 0:1])
        nc.scalar.activation(bscr2[:, :], bscr[:, :], AF.Exp, scale=-0.375)
        nc.scalar.activation(bscr[:, :], bscr2[:, :], AF.Ln, bias=0.25, scale=0.5,
                             accum_out=lgb_acc[:, 0:1])
        nc.scalar.activation(bscr2[:, :], epsb[:, :], AF.Square,
                             accum_out=sqb_acc[:, 0:1])
        assert R == 2
        nc.vector.tensor_tensor(
            biash[:, :], biasb[:, 0 : BRF : 2], biasb[:, 1 : BRF : 2], OP.add
        )
        nc.scalar.mul(biash[:, :], biash[:, :], 0.5)
        nc.sync.dma_start(biash0[:, :], biash[:, :])
        # biash0 += x @ mu  (accumulating SWDGE DMA, once)
        nc.gpsimd.dma_start(biash0[:, :], xmu_sb[:, :], accum_op=OP.add)

        # ---------------- main loop over half-slab units ----------------
        first_gram = True
        for b in range(BPC):
            ps_b = psmv.tile([1, OUT], f32, tag="mv", name="ps_b")
            for r in range(R):
                s8 = b * R + r
                for h in range(NH):
                    u = (b * R + r) * NH + h
                    gt0 = h * HT
                    hsl = slice(gt0 * OUT, (gt0 + HT) * OUT)
                    ep = epsp.tile([128, HF], f32r, tag="ep", name="ep")
                    nc.sync.dma_start(
                        ep[:, :].rearrange("p (t o) -> p t o", t=HT),
                        eps_d.ap()[s8, gt0 : gt0 + HT].rearrange("t p o -> p t o"),
                    )
                    s = sqp.tile([128, HF], f32r, tag="sq", name="s")
                    nc.vector.tensor_mul(
                        s[:, :], ep[:, :].bitcast(f32), sig[:, hsl]
                    )
                    w = chA.tile([128, HF], f32r, tag="chA", name="w")
                    nc.vector.tensor_add(
                        w[:, :], s[:, :].bitcast(f32),
                        mu[:, hsl],
                    )
                    q = sqp.tile([128, HF], f32, tag="sq", name="q")
                    nc.gpsimd.tensor_tensor(
                        q[:, :], w[:, :].bitcast(f32), w[:, :].bitcast(f32), OP.mult
                    )
                    # e = exp(-3q/8) in place, then ln(0.5e+0.25) -> accum
                    nc.scalar.activation(q[:, :], q[:, :], AF.Exp, scale=-0.375)
                    nc.scalar.activation(
                        dummy[:, :].to_broadcast((128, HF)), q[:, :], AF.Ln,
                        bias=0.25, scale=0.5, accum_out=acclg[:, u : u + 1],
                    )
                    # PE: the two Grams + matvec
                    st0 = first_gram
                    first_gram = False
                    nch = HF // 128
                    for c in range(nch):
                        g = c // GK
                        last = (u == U - 1) and (c == nch - 1)
                        nc.tensor.matmul(
                            gram_w[:, :],
                            lhsT=w[:, c * 128 : (c + 1) * 128],
                            rhs=w[:, g * GN : (g + 1) * GN],
                            start=st0 and (c == 0),
                            stop=last,
                        )
                        nc.tensor.matmul(
                            gram_e[:, :],
                            lhsT=ep[:, c * 128 : (c + 1) * 128],
                            rhs=ep[:, g * GN : (g + 1) * GN],
                            start=st0 and (c == 0),
                            stop=last,
                        )
                    for lt in range(HT):
                        gt = gt0 + lt
                        for oh in range(OH):
                            last = (r == R - 1) and (h == NH - 1) and (lt == HT - 1)
                            if mv_f32r:
                                mv_l = xT[:, gt * BPC + b : gt * BPC + b + 1]
                                mv_r = s[:, lt * OUT + oh * ON : lt * OUT + (oh + 1) * ON]
                            else:
                                mv_l = xT[:, gt * BPC + b : gt * BPC + b + 1].bitcast(f32)
                                mv_r = s[:, lt * OUT + oh * ON : lt * OUT + (oh + 1) * ON].bitcast(f32)
                            nc.tensor.matmul(
                                ps_b[0:1, oh * ON : (oh + 1) * ON],
                                lhsT=mv_l,
                                rhs=mv_r,
                                start=(r == 0 and h == 0 and lt == 0),
                                stop=last,
                            )
            # out_b = 0.5*psum + (bias_half + x@mu)   (in place over biash0)
            nc.vector.scalar_tensor_tensor(
                out=biash0[0:1, b * OUT : (b + 1) * OUT],
                in0=ps_b[0:1, :],
                scalar=0.5,
                in1=biash0[0:1, b * OUT : (b + 1) * OUT],
                op0=OP.mult,
                op1=OP.add,
            )

        # ---------------- final reductions ----------------
        scr = const.tile([128, GN], f32)
        nc.vector.affine_mul_reduce(
            out=scr[:, :], accum_out=acc_stack[:, S_QW : S_QW + 1],
            in0=gram_w[:, :], in1=mask[:, :], scale=1.0, bias=0.0,
        )
        nc.vector.affine_mul_reduce(
            out=scr[:, :], accum_out=acc_stack[:, S_SQW : S_SQW + 1],
            in0=gram_e[:, :], in1=mask[:, :], scale=1.0, bias=0.0,
        )
        nc.vector.tensor_reduce(
            acc_stack[:, S_LGW : S_LGW + 1], acclg[:, :], mybir.AxisListType.X, OP.add
        )
        nc.vector.tensor_reduce(
            acc_stack[:, S_LSW : S_LSW + 1], accls[:, :], mybir.AxisListType.X, OP.add
        )
        nc.vector.tensor_copy(acc_stack[:, S_QB : S_QB + 1], qb_acc[:, :])
        nc.vector.tensor_copy(acc_stack[:, S_LGB : S_LGB + 1], lgb_acc[:, :])
        nc.vector.tensor_copy(acc_stack[:, S_SQB : S_SQB + 1], sqb_acc[:, :])
        nc.vector.tensor_copy(acc_stack[:, S_LSB : S_LSB + 1], lsb_acc[:, :])

        pss = pssc.tile([8, 1], f32)
        nc.tensor.matmul(
            pss[:, :], lhsT=acc_stack[:, :], rhs=ones[:, :], start=True, stop=True
        )
        nc.vector.memset(stats_sb[:, :], 0.0)
        nc.scalar.copy(stats_sb[0:8, :], pss[:, :])

        nc.sync.dma_start(stats_d.ap(), stats_sb[:, :])
        nc.sync.dma_start(out_d.ap(), biash0[:, :])

    nc.compile()
    return nc


_NC_CACHE = {}


def _get_nc():
    key = "full"
    if key not in _NC_CACHE:
        _NC_CACHE[key] = build_kernel()
    return _NC_CACHE[key]


def make_in_maps(x, mu, ro, mu_bias, ro_bias, eps, eps_bias, n_cores=N_CORES,
                 gram_n=256):
    B, Rr, IN, OUT = eps.shape
    BPC = B // n_cores
    IT = IN // 128
    maskv = np.tile(np.eye(128, dtype=np.float32), (1, gram_n // 128))
    maskv = np.ascontiguousarray(maskv)
    in_maps = []
    for c in range(n_cores):
        bs = slice(c * BPC, (c + 1) * BPC)
        xt = x[bs].T.reshape(IT, 128, BPC).transpose(1, 0, 2).reshape(128, IT * BPC)
        eb = eps_bias[bs].transpose(0, 2, 1).reshape(128, -1)
        in_maps.append(
            {
                "eps": np.ascontiguousarray(eps[bs].reshape(BPC * Rr, IT, 128, OUT)),
                "xT": np.ascontiguousarray(xt),
                "xTf": np.ascontiguousarray(xt),
                "mu": np.ascontiguousarray(mu.reshape(IT, 128, OUT)),
                "ro": np.ascontiguousarray(ro.reshape(IT, 128, OUT)),
                "mu_bias": np.ascontiguousarray(mu_bias),
                "ro_bias": np.ascontiguousarray(ro_bias),
                "eps_bias": np.ascontiguousarray(eb),
                "diagmask": maskv,
            }
        )
    return in_maps


def combine_outputs(results, B, Rr, IN, OUT, n_cores=N_CORES):
    BPC = B // n_cores
    out = np.concatenate(
        [r["out"].reshape(BPC, OUT) for r in results], axis=0
    ).astype(np.float32)

    st = np.stack([r["stats"].reshape(16)[:8].astype(np.float64) for r in results])
    n_w_tot = float(B * Rr * IN * OUT)
    n_b_tot = float(B * Rr * OUT)
    denom = float(B * Rr)

    sum_q_w = st[:, S_QW].sum()
    sum_lg_w = st[:, S_LGW].sum()
    sum_sq_w = st[:, S_SQW].sum()
    sum_ls_w = st[0, S_LSW]
    sum_q_b = st[:, S_QB].sum()
    sum_lg_b = st[:, S_LGB].sum()
    sum_sq_b = st[:, S_SQB].sum()
    sum_ls_b = st[0, S_LSB]

    mean_lnmix_w = (sum_lg_w - sum_q_w / 8.0) / n_w_tot - LOG_SQRT_2PI
    mean_lnmix_b = (sum_lg_b - sum_q_b / 8.0) / n_b_tot - LOG_SQRT_2PI
    log_prior = (mean_lnmix_w + mean_lnmix_b) / denom

    mean_lpw_w = -sum_sq_w / (2.0 * n_w_tot) - sum_ls_w / (IN * OUT) - LOG_SQRT_2PI
    mean_lpw_b = -sum_sq_b / (2.0 * n_b_tot) - sum_ls_b / OUT - LOG_SQRT_2PI
    log_p_weights = (mean_lpw_w + mean_lpw_b) / denom

    return out, np.float32(log_prior), np.float32(log_p_weights)


def kernel(x, mu, ro, mu_bias, ro_bias, eps, eps_bias, trace=False):
    from concourse.bass_utils import run_bass_kernel_spmd

    x = np.asarray(x, dtype=np.float32)
    mu = np.asarray(mu, dtype=np.float32)
    ro = np.asarray(ro, dtype=np.float32)
    mu_bias = np.asarray(mu_bias, dtype=np.float32)
    ro_bias = np.asarray(ro_bias, dtype=np.float32)
    eps = np.asarray(eps, dtype=np.float32)
    eps_bias = np.asarray(eps_bias, dtype=np.float32)

    B, Rr, IN, OUT = eps.shape
    nc = _get_nc()
    in_maps = make_in_maps(x, mu, ro, mu_bias, ro_bias, eps, eps_bias)
    res = run_bass_kernel_spmd(
        nc, in_maps, core_ids=list(range(N_CORES)), trace=trace
    )
    out, log_prior, log_p = combine_outputs(res.results, B, Rr, IN, OUT)
    kernel.last_results = res
    return out, log_prior, log_p


# revision 33
# speedup vs baseline: 6.7135x; 6.7135x over previous
"""Trainium2 Bass kernel for nn_BayesianLayer (dense_mlp).

Reference computation (B=32, R=2, IN=OUT=1024):
    sigma      = softplus(ro)                      # (IN, OUT)
    weights    = eps * sigma + mu                  # (B, R, IN, OUT)
    bias       = eps_b * softplus(ro_b) + mu_b     # (B, R, OUT)
    log_prior  = (mean(ln(mix(weights))) + mean(ln(mix(bias)))) / (B*R)
    log_p      = (mean(ln N(w; mu, sigma)) + mean(ln N(bias; mu_b, sigma_b))) / (B*R)
    out        = (einsum('bi,brio->bro', x, weights) + bias).mean(axis=1)

Device math (per element, w = eps*sigma + mu, q = w^2):
    ln mix(w)      = -q/8 - ln(sqrt(2pi)) + ln(0.5*exp(-3q/8) + 0.25)
    ln N(w;mu,sig) = -eps^2/2 - ln(sigma) - ln(sqrt(2pi))
Each core produces partial sums (Sum w^2, Sum ln(0.5e+0.25), Sum eps^2,
Sum ln sigma; same four for the bias) plus its 4 rows of `out`; the host
does the final exact float64 combination (the "all-reduce" of the hint).

Sharding: data-parallel over batch. Core c handles batches [4c, 4c+4).

Engine split per half-slab unit (128 x 4096 elements):
    DVE:    s = eps*sigma (f32r out) ; w = s + mu (f32r out)
    GpSimd: q = w*w (separate tile, rotates with s)
    ACT:    e = exp(-3q/8) in place over q ; ln(0.5e+0.25) -> dummy, +accum
    PE:     matvec psum += x @ s (f32r); Gram(w) and Gram(eps) via f32r
            matmuls into two accumulating (128,256) psum tiles whose tiled
            diagonals hold per-column Sum w^2 / Sum eps^2 (extracted at the
            end with a host-provided tiled-identity mask);
            x @ mu once in exact fp32 (M=4), injected into the bias row via
            an accumulating GpSimd DMA.
"""

import sys

for _p in ("/opt/trn_rl_repo",):
    if _p not in sys.path:
        sys.path.insert(0, _p)

import math
from contextlib import ExitStack

import numpy as np

import concourse.bacc as bacc
import concourse.bass as bass
import concourse.mybir as mybir
import concourse.tile as tile

f32 = mybir.dt.float32
f32r = mybir.dt.float32r
AF = mybir.ActivationFunctionType
OP = mybir.AluOpType

N_CORES = 8
LOG_SQRT_2PI = 0.5 * math.log(2.0 * math.pi)

# Stats layout (partition index of the [16,1] stats output)
S_QW, S_LGW, S_SQW, S_LSW, S_QB, S_LGB, S_SQB, S_LSB = range(8)


def build_kernel(IN=1024, OUT=1024, BPC=4, R=2, HT=4, gram_n=256, mv_f32r=True):
    """Per-core DRAM inputs:
      eps      [BPC*R, IT, 128, OUT] (f32r bits = raw f32)
      xT       [128, IT*BPC] (f32r)     col t*BPC+b holds x[b, t*128+p]
      mu, ro   [IT, 128, OUT]           (mu declared f32r)
      mu_bias, ro_bias [1, OUT]
      eps_bias [128, BRF]               flat (b, o, r) order
      diagmask [128, gram_n]            tiled identity for Gram diag extract
    Outputs: out [1, BPC*OUT] ; stats [16, 1]
    """
    IT = IN // 128
    assert IT % HT == 0
    NH = IT // HT
    OH = max(1, OUT // 512)
    ON = OUT // OH
    U = BPC * R * NH
    HF = HT * OUT
    GN = gram_n                  # Gram rhs width (>=256 keeps f32r at 1 cyc/row)
    GK = GN // 128               # 128-chunks per Gram group

    RPB = 128 // BPC
    OPR = OUT // RPB
    BRF = OPR * R

    nc = bacc.Bacc("TRN2", target_bir_lowering=False, debug=False,
                   num_devices=N_CORES)

    _c = nc.alloc_sbuf_tensor("const-f32-qmix", [128, 1], f32)
    nc.gpsimd.memset(_c.ap(), 0.25)
    nc.const_aps.aps[(f32, 0.25)] = _c.ap()
    nc.all_engine_barrier()

    eps_d = nc.dram_tensor("eps", [BPC * R, IT, 128, OUT], f32r, kind="ExternalInput")
    xT_d = nc.dram_tensor("xT", [128, IT * BPC], f32r, kind="ExternalInput")
    xTf_d = nc.dram_tensor("xTf", [128, IT * BPC], f32, kind="ExternalInput")
    mu_d = nc.dram_tensor("mu", [IT, 128, OUT], f32, kind="ExternalInput")
    ro_d = nc.dram_tensor("ro", [IT, 128, OUT], f32, kind="ExternalInput")
    mub_d = nc.dram_tensor("mu_bias", [1, OUT], f32, kind="ExternalInput")
    rob_d = nc.dram_tensor("ro_bias", [1, OUT], f32, kind="ExternalInput")
    epsb_d = nc.dram_tensor("eps_bias", [128, BRF], f32, kind="ExternalInput")
    mask_d = nc.dram_tensor("diagmask", [128, GN], f32, kind="ExternalInput")

    out_d = nc.dram_tensor("out", [1, BPC * OUT], f32, kind="ExternalOutput")
    stats_d = nc.dram_tensor("stats", [16, 1], f32, kind="ExternalOutput")

    with tile.TileContext(nc) as tc, ExitStack() as ctx:
        const = ctx.enter_context(tc.tile_pool(name="const", bufs=1))
        sigmu = ctx.enter_context(tc.tile_pool(name="sigmu", bufs=1))
        epsp = ctx.enter_context(tc.tile_pool(name="epsp", bufs=2))
        sqp = ctx.enter_context(tc.tile_pool(name="sqp", bufs=3))
        chA = ctx.enter_context(tc.tile_pool(name="chA", bufs=2))
        psmv = ctx.enter_context(tc.tile_pool(name="psmv", bufs=2, space="PSUM"))
        psg = ctx.enter_context(tc.tile_pool(name="psg", bufs=1, space="PSUM"))
        pssc = ctx.enter_context(tc.tile_pool(name="pssc", bufs=1, space="PSUM"))

        # ---------------- persistent tiles ----------------
        sig = sigmu.tile([128, IT * OUT], f32)
        mu = sigmu.tile([128, IT * OUT], f32)

        acclg = const.tile([128, U], f32)
        accls = const.tile([128, 2], f32)
        acc_stack = const.tile([128, 8], f32)
        for _t in (acclg, accls, acc_stack):
            nc.vector.memset(_t[:, :], 0.0)

        xT = const.tile([128, IT * BPC], f32r)
        xTf = const.tile([128, IT * BPC], f32)
        ones = const.tile([128, 1], f32)
        nc.vector.memset(ones[:, :], 1.0)
        dummy = const.tile([128, 1], f32)
        mask = const.tile([128, GN], f32)
        nc.sync.dma_start(mask[:, :], mask_d.ap())

        biash0 = const.tile([1, BPC * OUT], f32)  # 0.5*bias_sum + x@mu, then out
        xmu_sb = const.tile([BPC, OUT], f32)
        stats_sb = const.tile([16, 1], f32)

        nc.sync.dma_start(xT[:, :], xT_d.ap())
        nc.sync.dma_start(xTf[:, :], xTf_d.ap())

        # Gram accumulators
        gram_w = psg.tile([128, GN], f32)
        gram_e = psg.tile([128, GN], f32)

        # ---------------- mu / sigma setup ----------------
        nc.sync.dma_start(
            mu[:, :].rearrange("p (t o) -> p t o", t=IT),
            mu_d.ap().rearrange("t p o -> p t o"),
        )
        for m in range(NH):
            sl = slice(m * HF, (m + 1) * HF)
            nc.sync.dma_start(
                sig[:, sl].rearrange("p (t o) -> p t o", t=HT),
                ro_d.ap()[m * HT : (m + 1) * HT].rearrange("t p o -> p t o"),
            )
            tmp = chA.tile([128, HF], f32, tag="chA", name="tmp")
            nc.scalar.activation(tmp[:, :], sig[:, sl], AF.Exp)
            nc.scalar.activation(sig[:, sl], tmp[:, :], AF.Ln, bias=1.0, scale=1.0)
        for m in range(2):
            sl = slice(m * (IT * OUT // 2), (m + 1) * (IT * OUT // 2))
            nc.scalar.activation(
                dummy[:, :].to_broadcast((128, IT * OUT // 2)),
                sig[:, sl], AF.Ln, accum_out=accls[:, m : m + 1],
            )

        # ---------------- x @ mu (exact fp32, M=BPC) ----------------
        ps_xmu = psmv.tile([BPC, OUT], f32, tag="mv", name="ps_xmu")
        for t in range(IT):
            for oh in range(OH):
                nc.tensor.matmul(
                    ps_xmu[:, oh * ON : (oh + 1) * ON],
                    lhsT=xTf[:, t * BPC : (t + 1) * BPC],
                    rhs=mu[:, t * OUT + oh * ON : t * OUT + (oh + 1) * ON],
                    start=(t == 0),
                    stop=(t == IT - 1),
                )
        nc.scalar.copy(xmu_sb[:, :], ps_xmu[:, :])

        # ---------------- bias chain ----------------
        epsb = const.tile([128, BRF], f32)
        sigb = const.tile([128, OPR], f32)
        mub = const.tile([128, OPR], f32)
        biasb = const.tile([128, BRF], f32)
        bscr = const.tile([128, BRF], f32)
        bscr2 = const.tile([128, BRF], f32)
        biash = const.tile([128, BRF // 2], f32)
        lsb_acc = const.tile([128, 1], f32)
        qb_acc = const.tile([128, 1], f32)
        lgb_acc = const.tile([128, 1], f32)
        sqb_acc = const.tile([128, 1], f32)
        for t_ in (lsb_acc, qb_acc, lgb_acc, sqb_acc):
            nc.vector.memset(t_[:, :], 0.0)
        nc.sync.dma_start(epsb[:, :], epsb_d.ap())
        for b in range(BPC):
            nc.sync.dma_start(
                sigb[b * RPB : (b + 1) * RPB, :],
                rob_d.ap().rearrange("one (c v) -> (one c) v", c=RPB),
            )
            nc.sync.dma_start(
                mub[b * RPB : (b + 1) * RPB, :],
                mub_d.ap().rearrange("one (c v) -> (one c) v", c=RPB),
            )
        nc.scalar.activation(bscr[:, 0:OPR], sigb[:, :], AF.Exp)
        nc.scalar.activation(sigb[:, :], bscr[:, 0:OPR], AF.Ln, bias=1.0, scale=1.0)
        lnsb_out = const.tile([128, OPR], f32)
        nc.scalar.activation(
            lnsb_out[0:RPB, :], sigb[0:RPB, :], AF.Ln,
            accum_out=lsb_acc[0:RPB, 0:1],
        )
        sigb_b = sigb[:, :].to_broadcast((128, OPR, R))
        mub_b = mub[:, :].to_broadcast((128, OPR, R))
        epsb_3 = epsb[:, :].rearrange("p (v d) -> p v d", d=R)
        biasb_3 = biasb[:, :].rearrange("p (v d) -> p v d", d=R)
        nc.vector.tensor_tensor(biasb_3, epsb_3, sigb_b, OP.mult)
        nc.vector.tensor_tensor(biasb_3, biasb_3, mub_b, OP.add)
        nc.scalar.activation(bscr[:, :], biasb[:, :], AF.Square,
                             accum_out=qb_acc[:, 0:1])
        nc.scalar.activation(bscr2[:, :], bscr[:, :], AF.Exp, scale=-0.375)
        nc.scalar.activation(bscr[:, :], bscr2[:, :], AF.Ln, bias=0.25, scale=0.5,
                             accum_out=lgb_acc[:, 0:1])
        nc.scalar.activation(bscr2[:, :], epsb[:, :], AF.Square,
                             accum_out=sqb_acc[:, 0:1])
        assert R == 2
        nc.vector.tensor_tensor(
            biash[:, :], biasb[:, 0 : BRF : 2], biasb[:, 1 : BRF : 2], OP.add
        )
        nc.scalar.mul(biash[:, :], biash[:, :], 0.5)
        nc.sync.dma_start(biash0[:, :], biash[:, :])
        # biash0 += x @ mu  (accumulating SWDGE DMA, once)
        nc.gpsimd.dma_start(biash0[:, :], xmu_sb[:, :], accum_op=OP.add)

        # ---------------- main loop over half-slab units ----------------
        first_gram = True
        for b in range(BPC):
            ps_b = psmv.tile([1, OUT], f32, tag="mv", name="ps_b")
            for r in range(R):
                s8 = b * R + r
                for h in range(NH):
                    u = (b * R + r) * NH + h
                    gt0 = h * HT
                    hsl = slice(gt0 * OUT, (gt0 + HT) * OUT)
                    ep = epsp.tile([128, HF], f32r, tag="ep", name="ep")
                    nc.sync.dma_start(
                        ep[:, :].rearrange("p (t o) -> p t o", t=HT),
                        eps_d.ap()[s8, gt0 : gt0 + HT].rearrange("t p o -> p t o"),
                    )
                    s = sqp.tile([128, HF], f32r, tag="sq", name="s")
                    nc.vector.tensor_mul(
                        s[:, :], ep[:, :].bitcast(f32), sig[:, hsl]
                    )
                    w = chA.tile([128, HF], f32r, tag="chA", name="w")
                    nc.vector.tensor_add(
                        w[:, :], s[:, :].bitcast(f32),
                        mu[:, hsl],
                    )
                    q = sqp.tile([128, HF], f32, tag="sq", name="q")
                    nc.gpsimd.tensor_tensor(
                        q[:, :], w[:, :].bitcast(f32), w[:, :].bitcast(f32), OP.mult
                    )
                    # e = exp(-3q/8) in place, then ln(0.5e+0.25) -> accum
                    nc.scalar.activation(q[:, :], q[:, :], AF.Exp, scale=-0.375)
                    nc.scalar.activation(
                        dummy[:, :].to_broadcast((128, HF)), q[:, :], AF.Ln,
                        bias=0.25, scale=0.5, accum_out=acclg[:, u : u + 1],
                    )
                    # PE: the two Grams + matvec
                    st0 = first_gram
                    first_gram = False
                    nch = HF // 128
                    for c in range(nch):
                        g = c // GK
                        last = (u == U - 1) and (c == nch - 1)
                        nc.tensor.matmul(
                            gram_w[:, :],
                            lhsT=w[:, c * 128 : (c + 1) * 128],
                            rhs=w[:, g * GN : (g + 1) * GN],
                            start=st0 and (c == 0),
                            stop=last,
                        )
                        nc.tensor.matmul(
                            gram_e[:, :],
                            lhsT=ep[:, c * 128 : (c + 1) * 128],
                            rhs=ep[:, g * GN : (g + 1) * GN],
                            start=st0 and (c == 0),
                            stop=last,
                        )
                    for lt in range(HT):
                        gt = gt0 + lt
                        for oh in range(OH):
                            last = (r == R - 1) and (h == NH - 1) and (lt == HT - 1)
                            if mv_f32r:
                                mv_l = xT[:, gt * BPC + b : gt * BPC + b + 1]
                                mv_r = s[:, lt * OUT + oh * ON : lt * OUT + (oh + 1) * ON]
                            else:
                                mv_l = xT[:, gt * BPC + b : gt * BPC + b + 1].bitcast(f32)
                                mv_r = s[:, lt * OUT + oh * ON : lt * OUT + (oh + 1) * ON].bitcast(f32)
                            nc.tensor.matmul(
                                ps_b[0:1, oh * ON : (oh + 1) * ON],
                                lhsT=mv_l,
                                rhs=mv_r,
                                start=(r == 0 and h == 0 and lt == 0),
                                stop=last,
                            )
            # out_b = 0.5*psum + (bias_half + x@mu)   (in place over biash0)
            nc.vector.scalar_tensor_tensor(
                out=biash0[0:1, b * OUT : (b + 1) * OUT],
                in0=ps_b[0:1, :],
                scalar=0.5,
                in1=biash0[0:1, b * OUT : (b + 1) * OUT],
                op0=OP.mult,
                op1=OP.add,
            )

        # ---------------- final reductions ----------------
        scr = const.tile([128, GN], f32)
        nc.vector.affine_mul_reduce(
            out=scr[:, :], accum_out=acc_stack[:, S_QW : S_QW + 1],
            in0=gram_w[:, :], in1=mask[:, :], scale=1.0, bias=0.0,
        )
        nc.vector.affine_mul_reduce(
            out=scr[:, :], accum_out=acc_stack[:, S_SQW : S_SQW + 1],
            in0=gram_e[:, :], in1=mask[:, :], scale=1.0, bias=0.0,
        )
        nc.vector.tensor_reduce(
            acc_stack[:, S_LGW : S_LGW + 1], acclg[:, :], mybir.AxisListType.X, OP.add
        )
        nc.vector.tensor_reduce(
            acc_stack[:, S_LSW : S_LSW + 1], accls[:, :], mybir.AxisListType.X, OP.add
        )
        nc.vector.tensor_copy(acc_stack[:, S_QB : S_QB + 1], qb_acc[:, :])
        nc.vector.tensor_copy(acc_stack[:, S_LGB : S_LGB + 1], lgb_acc[:, :])
        nc.vector.tensor_copy(acc_stack[:, S_SQB : S_SQB + 1], sqb_acc[:, :])
        nc.vector.tensor_copy(acc_stack[:, S_LSB : S_LSB + 1], lsb_acc[:, :])

        pss = pssc.tile([8, 1], f32)
        nc.tensor.matmul(
            pss[:, :], lhsT=acc_stack[:, :], rhs=ones[:, :], start=True, stop=True
        )
        nc.vector.memset(stats_sb[:, :], 0.0)
        nc.scalar.copy(stats_sb[0:8, :], pss[:, :])

        nc.sync.dma_start(stats_d.ap(), stats_sb[:, :])
        nc.sync.dma_start(out_d.ap(), biash0[:, :])

    nc.compile()
    return nc


_NC_CACHE = {}


def _get_nc():
    key = "full"
    if key not in _NC_CACHE:
        _NC_CACHE[key] = build_kernel()
    return _NC_CACHE[key]


def make_in_maps(x, mu, ro, mu_bias, ro_bias, eps, eps_bias, n_cores=N_CORES,
                 gram_n=256):
    B, Rr, IN, OUT = eps.shape
    BPC = B // n_cores
    IT = IN // 128
    maskv = np.tile(np.eye(128, dtype=np.float32), (1, gram_n // 128))
    maskv = np.ascontiguousarray(maskv)
    in_maps = []
    for c in range(n_cores):
        bs = slice(c * BPC, (c + 1) * BPC)
        xt = x[bs].T.reshape(IT, 128, BPC).transpose(1, 0, 2).reshape(128, IT * BPC)
        eb = eps_bias[bs].transpose(0, 2, 1).reshape(128, -1)
        in_maps.append(
            {
                "eps": np.ascontiguousarray(eps[bs].reshape(BPC * Rr, IT, 128, OUT)),
                "xT": np.ascontiguousarray(xt),
                "xTf": np.ascontiguousarray(xt),
                "mu": np.ascontiguousarray(mu.reshape(IT, 128, OUT)),
                "ro": np.ascontiguousarray(ro.reshape(IT, 128, OUT)),
                "mu_bias": np.ascontiguousarray(mu_bias),
                "ro_bias": np.ascontiguousarray(ro_bias),
                "eps_bias": np.ascontiguousarray(eb),
                "diagmask": maskv,
            }
        )
    return in_maps


def combine_outputs(results, B, Rr, IN, OUT, n_cores=N_CORES):
    BPC = B // n_cores
    out = np.concatenate(
        [r["out"].reshape(BPC, OUT) for r in results], axis=0
    ).astype(np.float32)

    st = np.stack([r["stats"].reshape(16)[:8].astype(np.float64) for r in results])
    n_w_tot = float(B * Rr * IN * OUT)
    n_b_tot = float(B * Rr * OUT)
    denom = float(B * Rr)

    sum_q_w = st[:, S_QW].sum()
    sum_lg_w = st[:, S_LGW].sum()
    sum_sq_w = st[:, S_SQW].sum()
    sum_ls_w = st[0, S_LSW]
    sum_q_b = st[:, S_QB].sum()
    sum_lg_b = st[:, S_LGB].sum()
    sum_sq_b = st[:, S_SQB].sum()
    sum_ls_b = st[0, S_LSB]

    mean_lnmix_w = (sum_lg_w - sum_q_w / 8.0) / n_w_tot - LOG_SQRT_2PI
    mean_lnmix_b = (sum_lg_b - sum_q_b / 8.0) / n_b_tot - LOG_SQRT_2PI
    log_prior = (mean_lnmix_w + mean_lnmix_b) / denom

    mean_lpw_w = -sum_sq_w / (2.0 * n_w_tot) - sum_ls_w / (IN * OUT) - LOG_SQRT_2PI
    mean_lpw_b = -sum_sq_b / (2.0 * n_b_tot) - sum_ls_b / OUT - LOG_SQRT_2PI
    log_p_weights = (mean_lpw_w + mean_lpw_b) / denom

    return out, np.float32(log_prior), np.float32(log_p_weights)


def kernel(x, mu, ro, mu_bias, ro_bias, eps, eps_bias, trace=False):
    from concourse.bass_utils import run_bass_kernel_spmd

    x = np.asarray(x, dtype=np.float32)
    mu = np.asarray(mu, dtype=np.float32)
    ro = np.asarray(ro, dtype=np.float32)
    mu_bias = np.asarray(mu_bias, dtype=np.float32)
    ro_bias = np.asarray(ro_bias, dtype=np.float32)
    eps = np.asarray(eps, dtype=np.float32)
    eps_bias = np.asarray(eps_bias, dtype=np.float32)

    B, Rr, IN, OUT = eps.shape
    nc = _get_nc()
    in_maps = make_in_maps(x, mu, ro, mu_bias, ro_bias, eps, eps_bias)
    res = run_bass_kernel_spmd(
        nc, in_maps, core_ids=list(range(N_CORES)), trace=trace
    )
    out, log_prior, log_p = combine_outputs(res.results, B, Rr, IN, OUT)
    kernel.last_results = res
    return out, log_prior, log_p
